# revision 22
# baseline (speedup 1.0000x reference)
"""Multi-head attention with RoPE (B=2, S=2048, H=16 heads, D=64) on 8 TRN2
NeuronCores, tensor-parallel over heads (2 heads/core); host sums the 8
rank-128 partial outputs.

Per core c (heads 2c, 2c+1), all matmul operands fp16 (fp32 PSUM accum):
  - qT/kT per batch [128, 2048] and v (natural [tok, d] layout + a ones
    column for the softmax denominator) from a shared fp16 x^T input. RoPE
    for BOTH batches is folded into the projection phase by interleaving the
    token chunks (order 0,4,1,5,2,6,3,7) so each batch-half finishes early;
    rot = partition-swap via SBUF->SBUF DMA (gpsimd queue), rot*sin on
    GpSimd, cos-mul+add on DVE. q pre-scaled by 1/sqrt(D)*W_SCALE on host.
  - Attention per (batch, 512-wide q chunk), 16 k-blocks each: the two
    heads' score matmuls (contraction 64) are issued back-to-back at PE
    row-groups 0-63 / 64-127 (tile_position auto-derived from the operands'
    base partition) so they stream CONCURRENTLY (~2x, probe-verified), into
    the two 512-col halves (= two PSUM banks) of ONE [128,1024] tile.
  - One 1024-wide exp then covers both heads. exp is split across engines
    to beat the 16.8M-elem/core exp wall: ScalarE ACT Exp (~1.04us/tile,
    11 k-blocks) and DVE (5 k-blocks: custom quartic poly in
    u = s*log2e*K/16 units then ^16 by 4 squarings, ~2.4us/tile). Scores
    are carried in u units (folded into the host-side Wq scale); ScalarE
    recovers e^s via the free affine scale on the ACT instruction.
  - PV: ctx[65,512] += [v|1].T @ P-half per head, pipelined 3 k-blocks
    behind scores. PSUM: 3x2 score banks + 2 ctx = 8.
  - Normalize: l row -> [128,4] scatter DMA, DVE reciprocal, DRAM-bounce
    broadcast (gpsimd queue so its long semaphore waits never block the
    ScalarE/sync sequencers), ctx*(1/l) on GpSimd into fp16 ctx_sb.
  - Output projection runs as a dense tail: per 128-token row block, two
    N=512 matmuls into a [128,1024] psum tile, one 1024-wide drain
    (alternating ScalarE/DVE), one full-row store (alternating sync/scalar
    HWDGE queues). Partial outputs stored fp16, summed fp32 on the host.

Engine-balance rationale: the kernel is jointly limited by the PE stream
(~341k cycles; clock oscillates 1.2-2.4GHz via the HAM activity gate, so
keeping the PE queue gapless matters more than instruction count) and by
exp throughput (ScalarE ~107G elem/s + DVE ~55G via the 2-op chain).
"""
import numpy as np
import ml_dtypes

import concourse.bass as bass
import concourse.mybir as mybir
import concourse.tile as tile
from concourse import bacc
from concourse.bass_utils import run_bass_kernel_spmd

F32 = mybir.dt.float32
F16 = mybir.dt.float16

B, S, HID = 2, 2048, 1024
NH, HD = 16, 64
T = B * S                  # 4096 tokens
NCORES = 8
HPC = NH // NCORES         # 2 heads per core
DPC = HPC * HD             # 128 context dims per core
ROPE_BASE = 10000.0

LN2 = float(np.log(2.0))
EXP_K = 0.2351161176314222
W_SCALE = float(EXP_K / (16.0 * LN2))
ACT_EXP_SCALE = float(16.0 * LN2 / EXP_K)  # ScalarE: exp(scale*u) = e^s
EXP_C0, EXP_C1, EXP_C2 = 3.1184983616533066, 4.34718537794368, 2.947519153435453

_CACHE = {}


def _register_dve_exp_ops():
    """Register the two custom DVE ops (idempotent across calls)."""
    import concourse.dve_ops as dve_ops
    from concourse.dve_ops import DveOp
    from concourse.dve_spec import (
        Spec, Src0, C0, C1, C2, One, sq, lower as dve_lower, _has_src1)
    from concourse.dve_uop import DveOpSpec

    if "EXPA_QUARTIC_ANT" in dve_ops._SUB_OPCODE_FOR_NAME:
        by_name = {op.name: op for op in dve_ops.OPS}
        return by_name["EXPA_QUARTIC_ANT"], by_name["EXPB_SQ4_ANT"]

    def _ref_a(in0, in1, s0, s1, imm2):
        u = in0.astype(np.float32)
        return ((((np.float32(s0) * u + np.float32(s1)) * u
                  + np.float32(s1)) * u + np.float32(imm2)) * u
                + np.float32(1.0))

    def _ref_b(in0, in1, s0, s1, imm2):
        g = in0.astype(np.float32)
        for _ in range(4):
            g = (g * g).astype(np.float32)
        return g

    op_a = DveOp(
        "EXPA_QUARTIC_ANT",
        Spec(body=(((Src0 * C0 + C1) * Src0 + C1) * Src0 + C2) * Src0 + One,
             reference=_ref_a),
        subdim=False, uops_sha={})
    op_b = DveOp(
        "EXPB_SQ4_ANT",
        Spec(body=sq(sq(sq(sq(Src0)))), reference=_ref_b),
        subdim=False, uops_sha={})
    for op in (op_a, op_b):
        dve_ops.OPS.append(op)
        dve_ops._SUB_OPCODE_FOR_NAME[op.name] = (
            dve_ops._CUSTOM_DVE_ROW_BASE + len(dve_ops.OPS) - 1)
        dve_ops.CUSTOM_DVE_SPECS[op.name] = op.spec
        for ver in ("v3", "v4"):
            su = DveOpSpec(
                name=op.name,
                opcode=dve_ops.get_dve_sub_opcode(op.name),
                uops=dve_lower(op.spec, ver=ver),
                rd1_en=_has_src1(op.spec))
            op.uops_sha[ver] = su.sha(ver)
    return op_a, op_b


def _build_program():
    nc = bacc.Bacc("TRN2", target_bir_lowering=False, debug=False)

    xT_d = nc.dram_tensor("xT16", [HID, T], F16, kind="ExternalInput")
    wq_d = nc.dram_tensor("wq", [128, HID], F16, kind="ExternalInput")
    wk_d = nc.dram_tensor("wk", [128, HID], F16, kind="ExternalInput")
    wv_d = nc.dram_tensor("wv", [128, HID], F16, kind="ExternalInput")
    wo_d = nc.dram_tensor("wo", [DPC, HID], F16, kind="ExternalInput")
    cos_d = nc.dram_tensor("cosf", [128, S], F16, kind="ExternalInput")
    sin_d = nc.dram_tensor("sins", [128, S], F16, kind="ExternalInput")
    out_d = nc.dram_tensor("out", [T, HID], F16, kind="ExternalOutput")
    rscr_d = nc.dram_tensor("rscr", [16, 512], F32)  # 1/l rows bounce

    with tile.TileContext(nc) as tc:
        _emit(nc, tc, xT_d, wq_d, wk_d, wv_d, wo_d, cos_d, sin_d, out_d,
              rscr_d)
    nc.compile()
    return nc


# DVE h1-exp k-blocks per chunk index (rest go to ScalarE)
def _dve_set(ci):
    if ci == 0:
        return {8, 14}
    if ci == 1:
        return {5, 11, 14}
    return {2, 5, 8, 11, 14}


def _emit(nc, tc, xT_d, wq_d, wk_d, wv_d, wo_d, cos_d, sin_d, out_d, rscr_d):
    import contextlib
    EXPA, EXPB = _register_dve_exp_ops()
    Exp = mybir.ActivationFunctionType.Exp
    Copy = mybir.ActivationFunctionType.Copy
    ctx = contextlib.ExitStack()
    with ctx:
        singles = ctx.enter_context(tc.tile_pool(name="singles", bufs=1))
        xpool = ctx.enter_context(tc.tile_pool(name="xpool", bufs=8))
        rotp = ctx.enter_context(tc.tile_pool(name="rotp", bufs=1))
        ppool = ctx.enter_context(tc.tile_pool(name="ppool", bufs=4))
        gpool = ctx.enter_context(tc.tile_pool(name="gpool", bufs=2))
        lpool = ctx.enter_context(tc.tile_pool(name="lpool", bufs=2))
        bpool = ctx.enter_context(tc.tile_pool(name="bpool", bufs=2))
        opool = ctx.enter_context(tc.tile_pool(name="opool", bufs=8))

        # ---- persistent SBUF ----
        wq_sb = singles.tile([128, 8, DPC], F16)
        wk_sb = singles.tile([128, 8, DPC], F16)
        wv_sb = singles.tile([128, 8, DPC], F16)
        wo_sb = singles.tile([128, HID], F16)
        cos_sb = singles.tile([128, S], F16)
        sin_sb = singles.tile([128, S], F16)
        qT = [singles.tile([128, S], F16, name=f"qT{b}") for b in range(B)]
        kT = [singles.tile([128, S], F16, name=f"kT{b}") for b in range(B)]
        v_all = singles.tile([128, 64, 65], F16)  # slot = h*32+b*16+kb
        ctx_sb = [singles.tile([128, S], F16, name=f"ctx{b}") for b in range(B)]

        # first x tile on sync, weights next so the first matmul starts early
        x00 = xpool.tile([128, 512], F16, tag="xt")
        nc.sync.dma_start(out=x00[:], in_=xT_d[0:128, 0:512])
        # split wq/wk loads (kc0 / kc1-3 / kc4-7) so early matmuls never wait
        for w_sb, w_d, q in ((wq_sb, wq_d, nc.scalar), (wk_sb, wk_d, nc.sync)):
            for lo, hi in ((0, 1), (1, 4), (4, 8)):
                dst = w_sb[:, lo:hi, :]
                q.dma_start(
                    out=bass.AP(tensor=dst.tensor, offset=dst.offset,
                                ap=[list(dst.ap[0]), [1, (hi - lo) * 128]]),
                    in_=w_d[:, lo * 128:hi * 128])
        nc.gpsimd.dma_start(out=wv_sb[:].rearrange("p a b -> p (a b)"),
                            in_=wv_d[:])
        nc.gpsimd.dma_start(out=cos_sb[:], in_=cos_d[:])
        nc.gpsimd.dma_start(out=sin_sb[:], in_=sin_d[:])
        nc.gpsimd.dma_start(out=wo_sb[:], in_=wo_d[:])
        nc.vector.memset(v_all[:, :, 64:65], 1.0)
        # preload the ACT exp table so the first real exp is cheap
        tblw = singles.tile([1, 8], F32)
        nc.vector.memset(tblw[:], 0.0)
        nc.scalar.activation(out=tblw[:], in_=tblw[:], func=Exp)

        rot_tiles = {}

        def rope_part(t_sb, t_name, b, half):
            cs = slice(half * 1024, (half + 1) * 1024)
            key = (t_name, b)
            if key not in rot_tiles:
                rot_tiles[key] = rotp.tile([128, S], F16, tag=f"rot{t_name}{b}",
                                           name=f"rot{t_name}{b}")
            rot = rot_tiles[key]
            nc.gpsimd.dma_start(out=rot[0:32, cs], in_=t_sb[32:64, cs])
            nc.gpsimd.dma_start(out=rot[32:64, cs], in_=t_sb[0:32, cs])
            nc.gpsimd.dma_start(out=rot[64:96, cs], in_=t_sb[96:128, cs])
            nc.gpsimd.dma_start(out=rot[96:128, cs], in_=t_sb[64:96, cs])
            nc.gpsimd.tensor_mul(rot[:, cs], rot[:, cs], sin_sb[:, cs])
            nc.vector.tensor_mul(t_sb[:, cs], t_sb[:, cs], cos_sb[:, cs])
            nc.vector.tensor_add(t_sb[:, cs], t_sb[:, cs], rot[:, cs])

        def rope_bh(b, half):
            rope_part(qT[b], "q", b, half)
            rope_part(kT[b], "k", b, half)

        # ---- phase 1: q/k (per-batch T-layout) + v (natural) ----
        TCN_ORDER = [0, 4, 1, 5, 2, 6, 3, 7]
        ROPES = {3: (0, 0), 5: (1, 0), 6: (0, 1), 7: (1, 1)}
        with tc.tile_pool(name="qkps", bufs=3, space="PSUM") as qkps, \
             tc.tile_pool(name="vps", bufs=2, space="PSUM") as vps:
            with nc.named_scope("qkv"):
                for pos, tcn in enumerate(TCN_ORDER):
                    bb, lsl = tcn // 4, slice((tcn % 4) * 512, (tcn % 4 + 1) * 512)
                    tsl = slice(tcn * 512, (tcn + 1) * 512)
                    psq = qkps.tile([128, 512], F32, tag="psq")
                    psk = qkps.tile([128, 512], F32, tag="psk")
                    # all 4 v sub-tiles packed in ONE psum bank (column
                    # slices): only the very first matmul clears the bank;
                    # the rest rely on per-element has_written bits.
                    pvt = vps.tile([128, 512], F32, tag="pv", name="pv")
                    pvs = [pvt[:, i * 128:(i + 1) * 128] for i in range(4)]
                    for kc in range(8):
                        if pos == 0 and kc == 0:
                            xt = x00
                        else:
                            xt = xpool.tile([128, 512], F16, tag="xt")
                            dma_eng = nc.sync if kc % 2 == 0 else nc.scalar
                            dma_eng.dma_start(
                                out=xt[:],
                                in_=xT_d[kc * 128:(kc + 1) * 128, tsl])
                        st, sp = kc == 0, kc == 7
                        nc.tensor.matmul(psq[:], wq_sb[:, kc, :], xt[:],
                                         start=st, stop=sp)
                        nc.tensor.matmul(psk[:], wk_sb[:, kc, :], xt[:],
                                         start=st, stop=sp)
                        for sub in range(4):
                            nc.tensor.matmul(
                                pvs[sub],
                                xt[:, sub * 128:(sub + 1) * 128],
                                wv_sb[:, kc, :],
                                start=(st and sub == 0), stop=sp,
                                skip_group_check=True)
                    for sub in range(4):
                        blk = tcn * 4 + sub
                        dst0 = v_all[:, blk, 0:64]
                        dst = bass.AP(tensor=dst0.tensor, offset=dst0.offset,
                                      ap=[list(dst0.ap[0]), [32 * 65, 2], [1, 64]])
                        if sub % 2 == 0:
                            nc.vector.tensor_copy(dst, pvs[sub])
                        else:
                            nc.scalar.activation(out=dst, in_=pvs[sub],
                                                 func=Copy)
                    nc.scalar.activation(out=qT[bb][:, lsl], in_=psq[:],
                                         func=Copy)
                    nc.vector.tensor_copy(kT[bb][:, lsl], psk[:])
                    if pos in ROPES:
                        rope_bh(*ROPES[pos])

        # ---- phase 2+3: attention; output projection runs as a dense tail ----
        projq = []

        def emit_proj_unit(pool, copy_eng=None, dma_eng=None, tag="pj"):
            # one unit = a full 128-token row block: two N=512 matmuls into a
            # 2-bank psum tile, one 1024-wide drain, one full-row store.
            bb, qb = projq.pop(0)
            qsl = slice(qb * 128, (qb + 1) * 128)
            ops = pool.tile([128, 1024], F32, tag=tag, name="ops")
            for oc in range(2):
                nc.tensor.matmul(ops[:, oc * 512:(oc + 1) * 512],
                                 ctx_sb[bb][:, qsl],
                                 wo_sb[:, oc * 512:(oc + 1) * 512],
                                 start=True, stop=True)
            ot = opool.tile([128, 1024], F16, tag="ot", name="ot")
            ce = copy_eng or nc.vector
            if ce is nc.scalar:
                nc.scalar.activation(out=ot[:], in_=ops[:], func=Copy)
            else:
                nc.vector.tensor_copy(ot[:], ops[:])
            de = dma_eng or nc.sync
            de.dma_start(
                out=out_d[bb * S + qb * 128:bb * S + (qb + 1) * 128, :],
                in_=ot[:])

        CHUNKS = [(0, 0), (0, 1), (0, 2), (0, 3),
                  (1, 0), (1, 1), (1, 2), (1, 3)]
        with tc.tile_pool(name="aps", bufs=3, space="PSUM") as aps, \
             tc.tile_pool(name="cps", bufs=1, space="PSUM") as cps:
            for ci, (b, qc) in enumerate(CHUNKS):
                dset = _dve_set(ci)
                qsl = slice(qc * 512, (qc + 1) * 512)
                with nc.named_scope(f"attn{ci}"):
                    ctxh = [cps.tile([65, 512], F32, tag=f"ctx{h}",
                                     name=f"ctx{h}")
                            for h in range(2)]
                    pring = {}
                    for kb in range(19):
                        if kb < 16:
                            ksl = slice(kb * 128, (kb + 1) * 128)
                            # both heads' scores into one 2-bank tile so a
                            # single 1024-wide exp covers the pair; the two
                            # row-group matmuls still run concurrently and
                            # drain to different banks (512-col halves).
                            spt = aps.tile([128, 1024], F32, tag="spp",
                                           name="spp")
                            for h in range(2):
                                rb = h * 64
                                nc.tensor.matmul(
                                    spt[:, h * 512:(h + 1) * 512],
                                    kT[b][rb:rb + 64, ksl],
                                    qT[b][rb:rb + 64, qsl],
                                    start=True, stop=True)
                            pt = ppool.tile([128, 1024], F16, tag="p",
                                            name="p")
                            if kb in dset:
                                gt = gpool.tile([128, 1024], F16, tag="g",
                                                name="g")
                                nc.vector._custom_dve(
                                    EXPA, out=gt[:], in0=spt[:],
                                    s0=EXP_C0, s1=EXP_C1, imm2=EXP_C2)
                                nc.vector._custom_dve(
                                    EXPB, out=pt[:], in0=gt[:])
                            else:
                                nc.scalar.activation(
                                    out=pt[:], in_=spt[:],
                                    func=Exp, scale=ACT_EXP_SCALE)
                            pring[kb] = pt
                        if kb >= 3:
                            kv = kb - 3
                            pt = pring.pop(kv)
                            st, sp = kv == 0, kv == 15
                            for h in range(2):
                                slot = h * 32 + b * 16 + kv
                                nc.tensor.matmul(
                                    ctxh[h][:], v_all[:, slot, 0:65],
                                    pt[:, h * 512:(h + 1) * 512],
                                    start=st, stop=sp)
                    # ---- drain + normalize both heads ----
                    nq = nc.gpsimd
                    for h in range(2):
                        rb = h * 64
                        cuh = lpool.tile([65, 512], F32, tag=f"cu{h}",
                                         name=f"cu{h}")
                        if h == 0:
                            nc.vector.tensor_copy(cuh[:], ctxh[h][:])
                        else:
                            nc.scalar.activation(out=cuh[:], in_=ctxh[h][:],
                                                 func=Copy)
                        idx = ci * 2 + h
                        lcol = lpool.tile([128, 4], F32, tag="lcol")
                        l0 = cuh[64:65, :]
                        nq.dma_start(
                            out=lcol[:],
                            in_=bass.AP(tensor=l0.tensor, offset=l0.offset,
                                        ap=[list(l0.ap[0]), [4, 128], [1, 4]]))
                        nc.vector.reciprocal(lcol[:], lcol[:])
                        r0 = rscr_d[idx, :]
                        nq.dma_start(
                            out=bass.AP(tensor=r0.tensor, offset=r0.offset,
                                        ap=[[4, 128], [1, 4]]),
                            in_=lcol[:])
                        bct = bpool.tile([64, 512], F32, tag="bct")
                        nq.dma_start(
                            out=bct[:],
                            in_=bass.AP(tensor=r0.tensor, offset=r0.offset,
                                        ap=[[0, 64], [1, 512]]))
                        nc.gpsimd.tensor_mul(
                            ctx_sb[b][rb:rb + 64, qsl], cuh[0:64, :], bct[:])
                    for qb in range(qc * 4, (qc + 1) * 4):
                        projq.append((b, qb))

        with tc.tile_pool(name="tps", bufs=4, space="PSUM") as tps:
            with nc.named_scope("projtail"):
                i = 0
                while projq:
                    emit_proj_unit(
                        tps,
                        copy_eng=nc.vector if i % 2 == 0 else nc.scalar,
                        dma_eng=(nc.sync, nc.scalar)[i % 2],
                        tag="tp")
                    i += 1


def _swz(w):
    # [1024, 128] -> [128, 1024]: SBUF layout [p, kc*128+d] = w[kc*128+p, d]
    return np.ascontiguousarray(
        w.reshape(8, 128, 128).transpose(1, 0, 2).reshape(128, 1024))


def _prep_inputs(x, Wq, Wk, Wv, Wo):
    x2 = np.asarray(x, dtype=np.float32).reshape(T, HID)
    xT16 = np.ascontiguousarray(x2.T).astype(np.float16)

    half = HD // 2
    inv_freq = (1.0 / (ROPE_BASE ** (np.arange(half, dtype=np.float64) * 2.0 / HD)))
    ang = np.arange(S, dtype=np.float64)[None, :] * inv_freq[:, None]  # [32, S]
    cosf = np.tile(np.cos(ang), (4, 1)).astype(np.float16)
    sgn = np.repeat([-1.0, 1.0, -1.0, 1.0], 32)[:, None]
    sins = (np.tile(np.sin(ang), (4, 1)) * sgn).astype(np.float16)

    scale = np.float32(1.0 / np.sqrt(HD)) * np.float32(W_SCALE)
    in_maps = []
    for c in range(NCORES):
        rows = slice(c * DPC, (c + 1) * DPC)
        in_maps.append({
            "xT16": xT16,
            "wq": _swz((Wq[rows, :] * scale).T.astype(np.float16)),
            "wk": _swz(Wk[rows, :].T.astype(np.float16)),
            "wv": _swz(Wv[rows, :].T.astype(np.float16)),
            "wo": np.ascontiguousarray(Wo[:, rows].T).astype(np.float16),
            "cosf": cosf,
            "sins": sins,
        })
    return in_maps


def _run(in_maps, trace=False):
    if "nc" not in _CACHE:
        _CACHE["nc"] = _build_program()
    nc = _CACHE["nc"]
    res = run_bass_kernel_spmd(nc, in_maps, core_ids=list(range(NCORES)),
                               trace=trace)
    acc = res.results[0]["out"].astype(np.float32).copy()
    for c in range(1, NCORES):
        acc += res.results[c]["out"]
    return acc.reshape(B, S, HID), res


def kernel(x, Wq, Wk, Wv, Wo):
    in_maps = _prep_inputs(np.asarray(x), np.asarray(Wq), np.asarray(Wk),
                           np.asarray(Wv), np.asarray(Wo))
    out, _ = _run(in_maps, trace=False)
    return out


def run_profiled(x, Wq, Wk, Wv, Wo):
    in_maps = _prep_inputs(np.asarray(x), np.asarray(Wq), np.asarray(Wk),
                           np.asarray(Wv), np.asarray(Wo))
    return _run(in_maps, trace=True)


# revision 23
# speedup vs baseline: 1.0249x; 1.0249x over previous
"""Multi-head attention with RoPE (B=2, S=2048, H=16 heads, D=64) on 8 TRN2
NeuronCores, tensor-parallel over heads (2 heads/core); host sums the 8
rank-128 partial outputs.

Per core c (heads 2c, 2c+1), all matmul operands fp16 (fp32 PSUM accum):
  - qT/kT per batch [128, 2048] and v (natural [tok, d] layout + a ones
    column for the softmax denominator) from a shared fp16 x^T input. RoPE
    for BOTH batches is folded into the projection phase by interleaving the
    token chunks (order 0,4,1,5,2,6,3,7) so each batch-half finishes early;
    rot = partition-swap via SBUF->SBUF DMA (gpsimd queue), rot*sin on
    GpSimd, cos-mul+add on DVE. q pre-scaled by 1/sqrt(D)*W_SCALE on host.
  - Attention per (batch, 512-wide q chunk), 16 k-blocks each: the two
    heads' score matmuls (contraction 64) are issued back-to-back at PE
    row-groups 0-63 / 64-127 (tile_position auto-derived from the operands'
    base partition) so they stream CONCURRENTLY (~2x, probe-verified), into
    the two 512-col halves (= two PSUM banks) of ONE [128,1024] tile.
  - One 1024-wide exp then covers both heads. exp is split across engines
    to beat the 16.8M-elem/core exp wall: ScalarE ACT Exp (~1.04us/tile,
    11 k-blocks) and DVE (5 k-blocks: custom quartic poly in
    u = s*log2e*K/16 units then ^16 by 4 squarings, ~2.4us/tile). Scores
    are carried in u units (folded into the host-side Wq scale); ScalarE
    recovers e^s via the free affine scale on the ACT instruction.
  - PV: ctx[65,512] += [v|1].T @ P-half per head, pipelined 3 k-blocks
    behind scores. PSUM: 3x2 score banks + 2 ctx = 8.
  - Normalize: l row -> [128,4] scatter DMA, DVE reciprocal, DRAM-bounce
    broadcast (gpsimd queue so its long semaphore waits never block the
    ScalarE/sync sequencers), ctx*(1/l) on GpSimd into fp16 ctx_sb.
  - Output projection runs as a dense tail: per 128-token row block, two
    N=512 matmuls into a [128,1024] psum tile, one 1024-wide drain
    (alternating ScalarE/DVE), one full-row store (alternating sync/scalar
    HWDGE queues). Partial outputs stored fp16, summed fp32 on the host.

Engine-balance rationale: the kernel is jointly limited by the PE stream
(~341k cycles; clock oscillates 1.2-2.4GHz via the HAM activity gate, so
keeping the PE queue gapless matters more than instruction count) and by
exp throughput (ScalarE ~107G elem/s + DVE ~55G via the 2-op chain).
"""
import numpy as np
import ml_dtypes

import concourse.bass as bass
import concourse.mybir as mybir
import concourse.tile as tile
from concourse import bacc
from concourse.bass_utils import run_bass_kernel_spmd

F32 = mybir.dt.float32
F16 = mybir.dt.float16

B, S, HID = 2, 2048, 1024
NH, HD = 16, 64
T = B * S                  # 4096 tokens
NCORES = 8
HPC = NH // NCORES         # 2 heads per core
DPC = HPC * HD             # 128 context dims per core
ROPE_BASE = 10000.0

LN2 = float(np.log(2.0))
EXP_K = 0.2351161176314222
W_SCALE = float(EXP_K / (16.0 * LN2))
ACT_EXP_SCALE = float(16.0 * LN2 / EXP_K)  # ScalarE: exp(scale*u) = e^s
EXP_C0, EXP_C1, EXP_C2 = 3.1184983616533066, 4.34718537794368, 2.947519153435453

_CACHE = {}


def _register_dve_exp_ops():
    """Register the two custom DVE ops (idempotent across calls)."""
    import concourse.dve_ops as dve_ops
    from concourse.dve_ops import DveOp
    from concourse.dve_spec import (
        Spec, Src0, C0, C1, C2, One, sq, lower as dve_lower, _has_src1)
    from concourse.dve_uop import DveOpSpec

    if "EXPA_QUARTIC_ANT" in dve_ops._SUB_OPCODE_FOR_NAME:
        by_name = {op.name: op for op in dve_ops.OPS}
        return by_name["EXPA_QUARTIC_ANT"], by_name["EXPB_SQ4_ANT"]

    def _ref_a(in0, in1, s0, s1, imm2):
        u = in0.astype(np.float32)
        return ((((np.float32(s0) * u + np.float32(s1)) * u
                  + np.float32(s1)) * u + np.float32(imm2)) * u
                + np.float32(1.0))

    def _ref_b(in0, in1, s0, s1, imm2):
        g = in0.astype(np.float32)
        for _ in range(4):
            g = (g * g).astype(np.float32)
        return g

    op_a = DveOp(
        "EXPA_QUARTIC_ANT",
        Spec(body=(((Src0 * C0 + C1) * Src0 + C1) * Src0 + C2) * Src0 + One,
             reference=_ref_a),
        subdim=False, uops_sha={})
    op_b = DveOp(
        "EXPB_SQ4_ANT",
        Spec(body=sq(sq(sq(sq(Src0)))), reference=_ref_b),
        subdim=False, uops_sha={})
    for op in (op_a, op_b):
        dve_ops.OPS.append(op)
        dve_ops._SUB_OPCODE_FOR_NAME[op.name] = (
            dve_ops._CUSTOM_DVE_ROW_BASE + len(dve_ops.OPS) - 1)
        dve_ops.CUSTOM_DVE_SPECS[op.name] = op.spec
        for ver in ("v3", "v4"):
            su = DveOpSpec(
                name=op.name,
                opcode=dve_ops.get_dve_sub_opcode(op.name),
                uops=dve_lower(op.spec, ver=ver),
                rd1_en=_has_src1(op.spec))
            op.uops_sha[ver] = su.sha(ver)
    return op_a, op_b


def _build_program():
    nc = bacc.Bacc("TRN2", target_bir_lowering=False, debug=False)

    xT_d = nc.dram_tensor("xT16", [HID, T], F16, kind="ExternalInput")
    wq_d = nc.dram_tensor("wq", [128, HID], F16, kind="ExternalInput")
    wk_d = nc.dram_tensor("wk", [128, HID], F16, kind="ExternalInput")
    wv_d = nc.dram_tensor("wv", [128, HID], F16, kind="ExternalInput")
    wo_d = nc.dram_tensor("wo", [DPC, HID], F16, kind="ExternalInput")
    cos_d = nc.dram_tensor("cosf", [128, S], F16, kind="ExternalInput")
    sin_d = nc.dram_tensor("sins", [128, S], F16, kind="ExternalInput")
    out_d = nc.dram_tensor("out", [T, HID], F16, kind="ExternalOutput")
    rscr_d = nc.dram_tensor("rscr", [16, 512], F32)  # 1/l rows bounce

    with tile.TileContext(nc) as tc:
        _emit(nc, tc, xT_d, wq_d, wk_d, wv_d, wo_d, cos_d, sin_d, out_d,
              rscr_d)
    nc.compile()
    return nc


# DVE h1-exp k-blocks per chunk index (rest go to ScalarE)
def _dve_set(ci):
    if ci == 0:
        return {8, 14}
    if ci == 1:
        return {5, 11, 14}
    return {2, 5, 8, 11, 14}


def _emit(nc, tc, xT_d, wq_d, wk_d, wv_d, wo_d, cos_d, sin_d, out_d, rscr_d):
    import contextlib
    EXPA, EXPB = _register_dve_exp_ops()
    Exp = mybir.ActivationFunctionType.Exp
    Copy = mybir.ActivationFunctionType.Copy
    ctx = contextlib.ExitStack()
    with ctx:
        singles = ctx.enter_context(tc.tile_pool(name="singles", bufs=1))
        xpool = ctx.enter_context(tc.tile_pool(name="xpool", bufs=12))
        rotp = ctx.enter_context(tc.tile_pool(name="rotp", bufs=1))
        ppool = ctx.enter_context(tc.tile_pool(name="ppool", bufs=4))
        gpool = ctx.enter_context(tc.tile_pool(name="gpool", bufs=2))
        lpool = ctx.enter_context(tc.tile_pool(name="lpool", bufs=2))
        bpool = ctx.enter_context(tc.tile_pool(name="bpool", bufs=2))
        opool = ctx.enter_context(tc.tile_pool(name="opool", bufs=8))

        # ---- persistent SBUF ----
        wq_sb = singles.tile([128, 8, DPC], F16)
        wk_sb = singles.tile([128, 8, DPC], F16)
        wv_sb = singles.tile([128, 8, DPC], F16)
        wo_sb = singles.tile([128, HID], F16)
        cos_sb = singles.tile([128, S], F16)
        sin_sb = singles.tile([128, S], F16)
        qT = [singles.tile([128, S], F16, name=f"qT{b}") for b in range(B)]
        kT = [singles.tile([128, S], F16, name=f"kT{b}") for b in range(B)]
        v_all = singles.tile([128, 64, 65], F16)  # slot = h*32+b*16+kb
        ctx_sb = [singles.tile([128, S], F16, name=f"ctx{b}") for b in range(B)]

        # first x tile on sync, weights next so the first matmul starts early
        x00 = xpool.tile([128, 512], F16, tag="xt")
        nc.sync.dma_start(out=x00[:], in_=xT_d[0:128, 0:512])
        # split wq/wk loads (kc0 / kc1-3 / kc4-7) so early matmuls never wait
        for w_sb, w_d, q in ((wq_sb, wq_d, nc.scalar), (wk_sb, wk_d, nc.sync)):
            for lo, hi in ((0, 1), (1, 4), (4, 8)):
                dst = w_sb[:, lo:hi, :]
                q.dma_start(
                    out=bass.AP(tensor=dst.tensor, offset=dst.offset,
                                ap=[list(dst.ap[0]), [1, (hi - lo) * 128]]),
                    in_=w_d[:, lo * 128:hi * 128])
        nc.gpsimd.dma_start(out=wv_sb[:].rearrange("p a b -> p (a b)"),
                            in_=wv_d[:])
        nc.gpsimd.dma_start(out=cos_sb[:], in_=cos_d[:])
        nc.gpsimd.dma_start(out=sin_sb[:], in_=sin_d[:])
        nc.gpsimd.dma_start(out=wo_sb[:], in_=wo_d[:])
        nc.vector.memset(v_all[:, :, 64:65], 1.0)
        tblw = singles.tile([1, 8], F32)
        nc.vector.memset(tblw[:], 0.0)

        rot_tiles = {}

        def rope_part(t_sb, t_name, b, half):
            cs = slice(half * 1024, (half + 1) * 1024)
            key = (t_name, b)
            if key not in rot_tiles:
                rot_tiles[key] = rotp.tile([128, S], F16, tag=f"rot{t_name}{b}",
                                           name=f"rot{t_name}{b}")
            rot = rot_tiles[key]
            nc.gpsimd.dma_start(out=rot[0:32, cs], in_=t_sb[32:64, cs])
            nc.gpsimd.dma_start(out=rot[32:64, cs], in_=t_sb[0:32, cs])
            nc.gpsimd.dma_start(out=rot[64:96, cs], in_=t_sb[96:128, cs])
            nc.gpsimd.dma_start(out=rot[96:128, cs], in_=t_sb[64:96, cs])
            nc.gpsimd.tensor_mul(rot[:, cs], rot[:, cs], sin_sb[:, cs])
            nc.vector.tensor_mul(t_sb[:, cs], t_sb[:, cs], cos_sb[:, cs])
            nc.vector.tensor_add(t_sb[:, cs], t_sb[:, cs], rot[:, cs])

        def rope_bh(b, half):
            rope_part(qT[b], "q", b, half)
            rope_part(kT[b], "k", b, half)

        # ---- phase 1: q/k (per-batch T-layout) + v (natural) ----
        TCN_ORDER = [0, 4, 1, 5, 2, 6, 3, 7]
        ROPES = {3: (0, 0), 5: (1, 0), 6: (0, 1), 7: (1, 1)}
        with tc.tile_pool(name="qkps", bufs=3, space="PSUM") as qkps, \
             tc.tile_pool(name="vps", bufs=2, space="PSUM") as vps:
            with nc.named_scope("qkv"):
                for pos, tcn in enumerate(TCN_ORDER):
                    bb, lsl = tcn // 4, slice((tcn % 4) * 512, (tcn % 4 + 1) * 512)
                    tsl = slice(tcn * 512, (tcn + 1) * 512)
                    psq = qkps.tile([128, 512], F32, tag="psq")
                    psk = qkps.tile([128, 512], F32, tag="psk")
                    # all 4 v sub-tiles packed in ONE psum bank (column
                    # slices): only the very first matmul clears the bank;
                    # the rest rely on per-element has_written bits.
                    pvt = vps.tile([128, 512], F32, tag="pv", name="pv")
                    pvs = [pvt[:, i * 128:(i + 1) * 128] for i in range(4)]
                    for kc in range(8):
                        if pos == 0 and kc == 0:
                            xt = x00
                        else:
                            xt = xpool.tile([128, 512], F16, tag="xt")
                            dma_eng = nc.sync if kc % 2 == 0 else nc.scalar
                            dma_eng.dma_start(
                                out=xt[:],
                                in_=xT_d[kc * 128:(kc + 1) * 128, tsl])
                        st, sp = kc == 0, kc == 7
                        nc.tensor.matmul(psq[:], wq_sb[:, kc, :], xt[:],
                                         start=st, stop=sp)
                        nc.tensor.matmul(psk[:], wk_sb[:, kc, :], xt[:],
                                         start=st, stop=sp)
                        for sub in range(4):
                            nc.tensor.matmul(
                                pvs[sub],
                                xt[:, sub * 128:(sub + 1) * 128],
                                wv_sb[:, kc, :],
                                start=(st and sub == 0), stop=sp,
                                skip_group_check=True)
                    for sub in range(4):
                        blk = tcn * 4 + sub
                        dst0 = v_all[:, blk, 0:64]
                        dst = bass.AP(tensor=dst0.tensor, offset=dst0.offset,
                                      ap=[list(dst0.ap[0]), [32 * 65, 2], [1, 64]])
                        if sub % 2 == 0:
                            nc.vector.tensor_copy(dst, pvs[sub])
                        else:
                            nc.scalar.activation(out=dst, in_=pvs[sub],
                                                 func=Copy)
                    nc.scalar.activation(out=qT[bb][:, lsl], in_=psq[:],
                                         func=Copy)
                    nc.vector.tensor_copy(kT[bb][:, lsl], psk[:])
                    if pos == 0:
                        # preload the ACT exp table here: late enough not to
                        # block the first x-tile DMA dispatches on the scalar
                        # queue, early enough to be resident before attention
                        nc.scalar.activation(out=tblw[:], in_=tblw[:],
                                             func=Exp)
                    if pos in ROPES:
                        rope_bh(*ROPES[pos])

        # ---- phase 2+3: attention; output projection runs as a dense tail ----
        projq = []

        def emit_proj_unit(pool, copy_eng=None, dma_eng=None, tag="pj"):
            # one unit = a full 128-token row block: two N=512 matmuls into a
            # 2-bank psum tile, one 1024-wide drain, one full-row store.
            bb, qb = projq.pop(0)
            qsl = slice(qb * 128, (qb + 1) * 128)
            ops = pool.tile([128, 1024], F32, tag=tag, name="ops")
            for oc in range(2):
                nc.tensor.matmul(ops[:, oc * 512:(oc + 1) * 512],
                                 ctx_sb[bb][:, qsl],
                                 wo_sb[:, oc * 512:(oc + 1) * 512],
                                 start=True, stop=True)
            ot = opool.tile([128, 1024], F16, tag="ot", name="ot")
            ce = copy_eng or nc.vector
            if ce is nc.scalar:
                nc.scalar.activation(out=ot[:], in_=ops[:], func=Copy)
            else:
                nc.vector.tensor_copy(ot[:], ops[:])
            de = dma_eng or nc.sync
            de.dma_start(
                out=out_d[bb * S + qb * 128:bb * S + (qb + 1) * 128, :],
                in_=ot[:])

        CHUNKS = [(0, 0), (0, 1), (0, 2), (0, 3),
                  (1, 0), (1, 1), (1, 2), (1, 3)]
        with tc.tile_pool(name="aps", bufs=3, space="PSUM") as aps, \
             tc.tile_pool(name="cps", bufs=1, space="PSUM") as cps:
            for ci, (b, qc) in enumerate(CHUNKS):
                dset = _dve_set(ci)
                qsl = slice(qc * 512, (qc + 1) * 512)
                with nc.named_scope(f"attn{ci}"):
                    ctxh = [cps.tile([65, 512], F32, tag=f"ctx{h}",
                                     name=f"ctx{h}")
                            for h in range(2)]
                    pring = {}
                    for kb in range(19):
                        if kb < 16:
                            ksl = slice(kb * 128, (kb + 1) * 128)
                            # both heads' scores into one 2-bank tile so a
                            # single 1024-wide exp covers the pair; the two
                            # row-group matmuls still run concurrently and
                            # drain to different banks (512-col halves).
                            spt = aps.tile([128, 1024], F32, tag="spp",
                                           name="spp")
                            for h in range(2):
                                rb = h * 64
                                nc.tensor.matmul(
                                    spt[:, h * 512:(h + 1) * 512],
                                    kT[b][rb:rb + 64, ksl],
                                    qT[b][rb:rb + 64, qsl],
                                    start=True, stop=True)
                            pt = ppool.tile([128, 1024], F16, tag="p",
                                            name="p")
                            if kb in dset:
                                gt = gpool.tile([128, 1024], F16, tag="g",
                                                name="g")
                                nc.vector._custom_dve(
                                    EXPA, out=gt[:], in0=spt[:],
                                    s0=EXP_C0, s1=EXP_C1, imm2=EXP_C2)
                                nc.vector._custom_dve(
                                    EXPB, out=pt[:], in0=gt[:])
                            else:
                                nc.scalar.activation(
                                    out=pt[:], in_=spt[:],
                                    func=Exp, scale=ACT_EXP_SCALE)
                            pring[kb] = pt
                        if kb >= 3:
                            kv = kb - 3
                            pt = pring.pop(kv)
                            st, sp = kv == 0, kv == 15
                            for h in range(2):
                                slot = h * 32 + b * 16 + kv
                                nc.tensor.matmul(
                                    ctxh[h][:], v_all[:, slot, 0:65],
                                    pt[:, h * 512:(h + 1) * 512],
                                    start=st, stop=sp)
                    # ---- drain + normalize both heads ----
                    nq = nc.gpsimd
                    for h in range(2):
                        rb = h * 64
                        cuh = lpool.tile([65, 512], F32, tag=f"cu{h}",
                                         name=f"cu{h}")
                        if h == 0:
                            nc.vector.tensor_copy(cuh[:], ctxh[h][:])
                        else:
                            nc.scalar.activation(out=cuh[:], in_=ctxh[h][:],
                                                 func=Copy)
                        idx = ci * 2 + h
                        lcol = lpool.tile([128, 4], F32, tag="lcol")
                        l0 = cuh[64:65, :]
                        nq.dma_start(
                            out=lcol[:],
                            in_=bass.AP(tensor=l0.tensor, offset=l0.offset,
                                        ap=[list(l0.ap[0]), [4, 128], [1, 4]]))
                        nc.vector.reciprocal(lcol[:], lcol[:])
                        r0 = rscr_d[idx, :]
                        nq.dma_start(
                            out=bass.AP(tensor=r0.tensor, offset=r0.offset,
                                        ap=[[4, 128], [1, 4]]),
                            in_=lcol[:])
                        bct = bpool.tile([64, 512], F32, tag="bct")
                        nq.dma_start(
                            out=bct[:],
                            in_=bass.AP(tensor=r0.tensor, offset=r0.offset,
                                        ap=[[0, 64], [1, 512]]))
                        nc.gpsimd.tensor_mul(
                            ctx_sb[b][rb:rb + 64, qsl], cuh[0:64, :], bct[:])
                    for qb in range(qc * 4, (qc + 1) * 4):
                        projq.append((b, qb))

        with tc.tile_pool(name="tps", bufs=4, space="PSUM") as tps:
            with nc.named_scope("projtail"):
                i = 0
                while projq:
                    emit_proj_unit(
                        tps,
                        copy_eng=nc.vector if i % 2 == 0 else nc.scalar,
                        dma_eng=(nc.sync, nc.scalar)[i % 2],
                        tag="tp")
                    i += 1


def _swz(w):
    # [1024, 128] -> [128, 1024]: SBUF layout [p, kc*128+d] = w[kc*128+p, d]
    return np.ascontiguousarray(
        w.reshape(8, 128, 128).transpose(1, 0, 2).reshape(128, 1024))


def _prep_inputs(x, Wq, Wk, Wv, Wo):
    x2 = np.asarray(x, dtype=np.float32).reshape(T, HID)
    xT16 = np.ascontiguousarray(x2.T).astype(np.float16)

    half = HD // 2
    inv_freq = (1.0 / (ROPE_BASE ** (np.arange(half, dtype=np.float64) * 2.0 / HD)))
    ang = np.arange(S, dtype=np.float64)[None, :] * inv_freq[:, None]  # [32, S]
    cosf = np.tile(np.cos(ang), (4, 1)).astype(np.float16)
    sgn = np.repeat([-1.0, 1.0, -1.0, 1.0], 32)[:, None]
    sins = (np.tile(np.sin(ang), (4, 1)) * sgn).astype(np.float16)

    scale = np.float32(1.0 / np.sqrt(HD)) * np.float32(W_SCALE)
    in_maps = []
    for c in range(NCORES):
        rows = slice(c * DPC, (c + 1) * DPC)
        in_maps.append({
            "xT16": xT16,
            "wq": _swz((Wq[rows, :] * scale).T.astype(np.float16)),
            "wk": _swz(Wk[rows, :].T.astype(np.float16)),
            "wv": _swz(Wv[rows, :].T.astype(np.float16)),
            "wo": np.ascontiguousarray(Wo[:, rows].T).astype(np.float16),
            "cosf": cosf,
            "sins": sins,
        })
    return in_maps


def _run(in_maps, trace=False):
    if "nc" not in _CACHE:
        _CACHE["nc"] = _build_program()
    nc = _CACHE["nc"]
    res = run_bass_kernel_spmd(nc, in_maps, core_ids=list(range(NCORES)),
                               trace=trace)
    acc = res.results[0]["out"].astype(np.float32).copy()
    for c in range(1, NCORES):
        acc += res.results[c]["out"]
    return acc.reshape(B, S, HID), res


def kernel(x, Wq, Wk, Wv, Wo):
    in_maps = _prep_inputs(np.asarray(x), np.asarray(Wq), np.asarray(Wk),
                           np.asarray(Wv), np.asarray(Wo))
    out, _ = _run(in_maps, trace=False)
    return out


def run_profiled(x, Wq, Wk, Wv, Wo):
    in_maps = _prep_inputs(np.asarray(x), np.asarray(Wq), np.asarray(Wk),
                           np.asarray(Wv), np.asarray(Wo))
    return _run(in_maps, trace=True)
